# revision 29
# baseline (speedup 1.0000x reference)
"""BTT layer (nn_BTTLayer_36885179138559) as a Trainium2 Bass kernel.

Factorized BTT evaluation, data-parallel over 8 cores (512 batch rows each).
Per batch-tile bt (128 rows):
  stage 1: inner[n, B, m*8+r] = x_n[B, 64] @ btt_r[n] -- issued as K=128
           matmuls where the moving operand (btt_r block) is zero-padded on
           the 64 partitions belonging to the sibling n-block, so every
           instruction runs in the uniform 128x128 PE mode (no tiling-mode
           switches) and stage-1 weight loads get Fast Weight Load. One
           2-bank PSUM pair tile per two n; a single eviction instruction
           scatters (m, r) into per-group S_g[B, m, .] (m-major butterfly).
  butterfly: PE transpose of S_g[:, m, :] -> psT -> Ts slabs (k=(n,r) on
           partitions); oct-outer/g-inner so only 8 Ts slabs are allocated.
  stage 2: ps2[B, m8, a] += Ts[g][:, m8, :]^T @ btt_l[m, g]; one PSUM
           has_written session per bank (start only on the first matmul --
           start=True clears has_written for the WHOLE bank).

Software pipelining: stage-1 matmuls for bt+1 are spread through bt's
transpose/stage-2 stream (2 after each transpose group), PSUM evictions run
on Scalar/Vector (cost-weighted choice) in the shadow. The zero halves of
the 4 rotating rt ring buffers are memset once and only data halves are
re-DMAed. S is double-buffered per g to fit SBUF.

Compute bf16 (fp32 PSUM), device output bf16, host casts to fp32.
"""

import numpy as np
import ml_dtypes

import concourse.bacc as bacc
import concourse.mybir as mybir
import concourse.tile as tile
import concourse.bass_utils as bass_utils

# problem dims (hardcoded per contract)
M, N, A, B_BLK, RANK = 64, 64, 64, 64, 8
D = 4096              # in = out features
ROWS = 4096           # flattened batch (4, 1024, 4096)
N_CORES = 8
BS = ROWS // N_CORES  # 512 rows per core
BT = 4                # batch tiles of 128

BF16 = mybir.dt.bfloat16
F32 = mybir.dt.float32

# butterfly transpose groups offloaded to the DMA XBAR vs kept on PE
# (XBAR transpose measured ~3x slower overall -- keep everything on PE)
DMA_T = ()
PE_T = (0, 1, 2, 3)

_compiled = None
_last_in_maps = None


def _build():
    nc = bacc.Bacc("TRN2", target_bir_lowering=False, debug=False, num_devices=N_CORES)
    # xtb[bt, p, t, c] = x_shard[row = 128*bt + c, feature = 128*t + p]
    xtb_ap = nc.dram_tensor("xtb", [BT, 128, 32, 128], BF16, kind="ExternalInput").ap()
    # rt[p, t, j] = btt_r[2t + p//64, p%64, j]
    rt_ap = nc.dram_tensor("rt", [128, 32, 512], BF16, kind="ExternalInput").ap()
    # lt[p, m, g, a] = btt_l[m, (16g + p//8)*8 + p%8, a]
    lt_ap = nc.dram_tensor("lt", [128, M, 4, A], BF16, kind="ExternalInput").ap()
    id_ap = nc.dram_tensor("ident", [128, 128], BF16, kind="ExternalInput").ap()
    # o[bt, oct, col, m8, a] bf16
    o_ap = nc.dram_tensor("o", [BT, 8, 128, 8, A], BF16, kind="ExternalOutput").ap()

    with tile.TileContext(nc) as tc:
        with (
            tc.tile_pool(name="wpool", bufs=1) as wpool,
            tc.tile_pool(name="xpool", bufs=1) as xpool,
            tc.tile_pool(name="spool", bufs=2) as spool,
            tc.tile_pool(name="tpool", bufs=2) as tpool,
            tc.tile_pool(name="opool", bufs=2) as opool,
            tc.tile_pool(name="ps1p", bufs=2, space="PSUM") as ps1p,
            tc.tile_pool(name="psTp", bufs=3, space="PSUM") as psTp,
            tc.tile_pool(name="ps2p", bufs=1, space="PSUM") as ps2p,
        ):
            XB = {}
            SG = {}
            TS = {}
            eng_ns = [0.0, 0.0]

            # 4 persistent rt ring buffers [128, 4 n-slots, 512]; slot s holds
            # the zero-padded block for n = 16g + 4q + s: data on partitions
            # 64*(s%2)..64*(s%2)+64, zeros elsewhere (memset once, reused).
            RTB = [wpool.tile([128, 4, 512], BF16, tag=f"rtb{i}", name=f"RTB{i}")
                   for i in range(4)]
            for i in range(4):
                nc.gpsimd.memset(RTB[i][:], 0.0)

            # global rt sub-chunk schedule in consumption order:
            # per era: g0q0, g2q0, g0q1, g2q1, ..., then g1/g3 interleaved
            rt_sched = []
            for bt in range(BT):
                for gpair in ((0, 2), (1, 3)):
                    for q in range(4):
                        for g in gpair:
                            rt_sched.append((bt, g, q))
            rt_loaded = [0]  # next schedule index to load

            def load_rt_next():
                i = rt_loaded[0]
                if i >= len(rt_sched):
                    return
                rt_loaded[0] += 1
                bt, g, q = rt_sched[i]
                buf = RTB[i % 4]
                # even slots (s=0,2): n even, data partitions 0-63
                # odd slots (s=1,3): n odd, data partitions 64-127
                t0 = 8 * g + 2 * q
                nc.sync.dma_start(buf[0:64, 0::2, :], rt_ap[0:64, t0:t0 + 2, :])
                nc.sync.dma_start(buf[64:128, 1::2, :], rt_ap[64:128, t0:t0 + 2, :])

            def rt_buf(i):
                return RTB[i % 4]

            rt_used = [0]  # next schedule index to consume

            # split by g-pair: xa holds feature tiles for g0/g2 (t 0-7,
            # 16-23), xb for g1/g3 (t 8-15, 24-31). xa(bt)'s WAR resolves
            # at oct3 of the phase before its use, so emitting xa early
            # pipelines without stalls; xb is only needed from oct4.
            def load_xa(bt):
                xa = xpool.tile([128, 16, 128], BF16, tag="xa", name=f"Xa{bt}")
                nc.sync.dma_start(xa[:, 0:8, :], xtb_ap[bt, :, 0:8, :])
                nc.sync.dma_start(xa[:, 8:16, :], xtb_ap[bt, :, 16:24, :])
                XB[bt] = [xa, None]

            def load_xb(bt):
                xb = xpool.tile([128, 16, 128], BF16, tag="xb", name=f"Xb{bt}")
                nc.sync.dma_start(xb[:, 0:8, :], xtb_ap[bt, :, 8:16, :])
                nc.sync.dma_start(xb[:, 8:16, :], xtb_ap[bt, :, 24:32, :])
                XB[bt][1] = xb

            def evict(dst, src, cost_s, cost_v):
                if eng_ns[0] + cost_s <= eng_ns[1] + cost_v:
                    nc.scalar.copy(dst, src)
                    eng_ns[0] += cost_s
                else:
                    nc.vector.tensor_copy(dst, src)
                    eng_ns[1] += cost_v

            def emit_subchunk_half(bt, g, q, j, ridx):
                # two K=128 stage-1 matmuls (n = 16g+4q+2j, +1) into a 2-bank
                # pair tile + one eviction into S_g
                if q == 0 and j == 0:
                    SG[(bt, g)] = spool.tile([128, M, 128], BF16, tag=f"S{g}",
                                             name=f"S_{bt}_{g}")
                xq = XB[bt][g % 2]            # xa for g0/g2, xb for g1/g3
                xslot = 8 * (g // 2) + 2 * q + j
                buf = rt_buf(ridx)
                nl = 4 * q + 2 * j    # n-loc of the even sibling
                ps1 = ps1p.tile([128, 2, 512], F32, tag="ps1",
                                name=f"ps1_{bt}_{g}_{nl}")
                nc.tensor.matmul(ps1[:, 0, :], xq[:, xslot, :], buf[:, 2 * j, :],
                                 start=True, stop=True)
                nc.tensor.matmul(ps1[:, 1, :], xq[:, xslot, :], buf[:, 2 * j + 1, :],
                                 start=True, stop=True)
                src = ps1[:].rearrange("p n (m r) -> p m n r", n=2, m=M, r=RANK)
                dst = SG[(bt, g)][:, :, 8 * nl:8 * nl + 16].rearrange(
                    "p m (n r) -> p m n r", n=2, r=RANK)
                evict(dst, src, 1150.0, 1250.0)

            rt_part = [0]

            def consume_next_half():
                # emit the next half of the current rt sub-chunk (2 MMs)
                i = rt_used[0]
                if i >= len(rt_sched):
                    return
                bt, g, q = rt_sched[i]
                emit_subchunk_half(bt, g, q, rt_part[0], i)
                if rt_part[0] == 0:
                    rt_part[0] = 1
                else:
                    rt_part[0] = 0
                    rt_used[0] += 1
                    ensure_rt(rt_ahead[0])

            rt_ahead = [2]  # rt load lookahead (2 in prologue, 4 steady)

            def ensure_rt(k):
                while rt_loaded[0] < min(rt_used[0] + k, len(rt_sched)):
                    load_rt_next()

            def t_phase(bt):
                nxt = bt + 1 if bt + 1 < BT else None
                rt_ahead[0] = 4
                ensure_rt(4)
                if nxt is not None:
                    load_xb(nxt)
                for oct in range(8):
                    # butterfly for this oct: groups in DMA_T go through the
                    # DMA XBAR transpose engine (issued early, SBUF->SBUF),
                    # the rest through PE transpose + psT eviction; next-bt
                    # stage-1 matmuls sprinkled after each PE transpose group
                    for g in DMA_T:
                        Sg = SG[(bt, g)]
                        Ts = tpool.tile([128, 8, 128], BF16, tag=f"T{g}",
                                        name=f"Ts_{bt}_{oct}_{g}")
                        for m8 in range(8):
                            nc.sync.dma_start_transpose(
                                Ts[:, m8, :], Sg[:, 8 * oct + m8, :])
                        TS[(g, oct)] = Ts
                    for gi, g in enumerate(PE_T):
                        psT = psTp.tile([128, 8, 128], BF16, tag="psT",
                                        name=f"psT_{bt}_{oct}_{g}")
                        Sg = SG[(bt, g)]
                        for m8 in range(8):
                            nc.tensor.transpose(psT[:, m8, :],
                                                Sg[:, 8 * oct + m8, :], ID[:])
                        Ts = tpool.tile([128, 8, 128], BF16, tag=f"T{g}",
                                        name=f"Ts_{bt}_{oct}_{g}")
                        evict(Ts[:], psT[:], 1000.0, 640.0)
                        TS[(g, oct)] = Ts
                        if nxt is not None:
                            consume_next_half()
                    # stage-2 for this oct; g-outer so the freshest Ts slab
                    # (g=3, evicted last) is consumed last. start=True clears
                    # has_written for the WHOLE bank -> only on first matmul.
                    ps2 = ps2p.tile([128, 8, A], F32, tag="ps2",
                                    name=f"ps2_{bt}_{oct}")
                    for g in range(4):
                        for m8 in range(8):
                            m = 8 * oct + m8
                            nc.tensor.matmul(ps2[:, m8, :], TS[(g, oct)][:, m8, :],
                                             LT[:, m, g, :],
                                             start=(g == 0 and m8 == 0),
                                             stop=(g == 3 and m8 == 7),
                                             skip_group_check=True)
                    if nxt is not None:
                        for _ in range(4 - len(PE_T)):
                            consume_next_half()
                        if oct == 5 and nxt + 1 < BT:
                            load_xa(nxt + 1)
                    osb = opool.tile([128, 8, A], BF16, tag="osb",
                                     name=f"osb_{bt}_{oct}")
                    evict(osb[:], ps2[:], 600.0, 640.0)
                    nc.sync.dma_start(o_ap[bt, oct], osb[:])
                for g in range(4):
                    SG.pop((bt, g), None)

            # prologue: first-needed DMAs first, then bt0 stage-1 up front;
            # rt lookahead stays at 2 here so era-1 streams don't congest
            # the head DMA queues
            load_xa(0)
            load_xb(0)
            ensure_rt(3)
            ID = wpool.tile([128, 128], BF16, tag="id", name="ID")
            nc.sync.dma_start(ID[:], id_ap)
            LT = wpool.tile([128, M, 4, A], BF16, tag="lt", name="LT")
            for sc in range(16):  # bt0's 16 sub-chunks
                consume_next_half()
                consume_next_half()
                if sc == 4:
                    for mq in range(4):
                        nc.sync.dma_start(LT[:, 16 * mq:16 * (mq + 1), :, :],
                                          lt_ap[:, 16 * mq:16 * (mq + 1), :, :])
                elif sc == 9:
                    load_xa(1)
                elif sc == 11:
                    rt_ahead[0] = 3
                    ensure_rt(3)
                elif sc == 13:
                    rt_ahead[0] = 4
                    ensure_rt(4)
            for bt in range(BT):
                t_phase(bt)
    nc.compile()
    return nc


def _get_compiled():
    global _compiled
    if _compiled is None:
        _compiled = _build()
    return _compiled


def kernel(x, btt_r, btt_l, bias):
    x = np.asarray(x)
    btt_r = np.asarray(btt_r)
    btt_l = np.asarray(btt_l)
    bias = np.asarray(bias)
    orig_shape = x.shape

    # rt[p, t, j] = btt_r[2t + p//64, p%64, j]
    rt = np.ascontiguousarray(
        btt_r.astype(np.float32).reshape(32, 2, 64, 512).transpose(1, 2, 0, 3)
        .reshape(128, 32, 512)
    ).astype(ml_dtypes.bfloat16)
    # lt[p, m, g, a] = btt_l[m, (16g + p//8)*8 + p%8, a]
    l4 = btt_l.astype(np.float32).reshape(M, 4, 16, RANK, A)
    lt = np.ascontiguousarray(l4.transpose(2, 3, 0, 1, 4).reshape(128, M, 4, A)
                              ).astype(ml_dtypes.bfloat16)
    ident = np.eye(128, dtype=ml_dtypes.bfloat16)

    # per-core x shards: xtb[bt, p, t, c] = xs[128*bt + c, 128*t + p]
    xr = x.astype(np.float32).reshape(ROWS, D)
    in_maps = []
    for c in range(N_CORES):
        xs = xr[c * BS:(c + 1) * BS]                               # (BS, D)
        xt = np.ascontiguousarray(
            xs.T.reshape(32, 128, BS).transpose(1, 0, 2)
        )                                                          # (128, 32, 512)
        xtb = np.ascontiguousarray(
            xt.reshape(128, 32, BT, 128).transpose(2, 0, 1, 3)
        ).astype(ml_dtypes.bfloat16)                               # (4, 128, 32, 128)
        in_maps.append({"xtb": xtb, "rt": rt, "lt": lt, "ident": ident})

    global _last_in_maps
    _last_in_maps = in_maps
    nc = _get_compiled()
    try:
        res = bass_utils.run_bass_kernel_spmd(nc, in_maps, core_ids=list(range(N_CORES)))
    except Exception:
        # transient device hiccups recover on retry
        import time as _time
        _time.sleep(10)
        res = bass_utils.run_bass_kernel_spmd(nc, in_maps, core_ids=list(range(N_CORES)))

    # gather: o[bt, oct, col, m8, a] -> rows (BS, D) per core
    out = np.empty((ROWS, D), dtype=np.float32)
    for c in range(N_CORES):
        o = np.asarray(res.results[c]["o"], dtype=np.float32)      # (4, 8, 128, 8, 64)
        out[c * BS:(c + 1) * BS] = o.transpose(0, 2, 1, 3, 4).reshape(BS, D)
    out += bias.astype(np.float32)[None, :]
    return out.reshape(*orig_shape[:-1], D)
